# revision 1
# baseline (speedup 1.0000x reference)
"""CVRP decoder kernel for 8 Trainium2 NeuronCores (pure data parallel).

Computes, per batch b:
    k = enc @ Wk.T ; v = enc @ Wv.T ; q = [eln, load] @ Wq.T
    eb = exp(-a1*ls*cur_dist + mask)
    weighted = (eb @ (exp(k)*v)) / (eb @ exp(k))
    aafm = sigmoid(q) * weighted
    score = aafm @ enc.T
    probs = softmax(10*tanh(score/sqrt(D) - a2*ls*cur_dist) + mask)

Sharding: batch (128) split across 8 cores, 16 batches/core. Weights are
replicated. Each core runs an identical Bass program (SPMD, no collectives).

Key layout decisions (driven by the matmul contract-on-partition rule):
  - enc/eln are transposed on the HOST (free) so all contractions over d
    have d on partitions.
  - eb is needed with m on partitions: cur_dist and mask/(-c1) (the host
    pre-divides the mask) are PE-transposed per 128x128 tile, accumulating
    cdT + maskT' in PSUM; ACT exp with per-partition scale -c1 then gives
    exp(-c1*cdT + maskT) on the way to SBUF.
  - score/softmax stay in natural (n, m) layout, so the softmax reduce is a
    free-dim reduction; the -sqrt(D)*c2*cur_dist term is folded into the
    score PSUM accumulation as an extra matmul against a scaled identity.
  - All matmuls use float32r (full PE rate at free >= 256); walrus requires
    matmul operands to be *produced* as f32r, so the DMA'd tensors are
    declared f32r end-to-end and the on-chip producers (ACT exp, DVE muls)
    write f32r directly. f32 views via bitcast where ACT/DVE read them.
  - Only activation functions from the "exp_and_others" table set are used
    (exp, tanh) so the ~2.7us table load happens exactly once:
    sigmoid(x) = 0.5 + 0.5*tanh(x/2); reciprocals go to the vector engine.
  - alpha1/alpha2/log_scale enter only through uploaded data (scaled
    identity, per-partition scale vectors), so one compiled program serves
    any input values. Caveat: |alpha1*log_scale| is clamped to >=1e-20 when
    pre-dividing the mask; exact whenever alpha1*log_scale is not
    vanishingly small or the mask is zero/-inf.
"""

import sys

if "/opt/trn_rl_repo" not in sys.path:
    sys.path.insert(0, "/opt/trn_rl_repo")

from contextlib import ExitStack

import numpy as np

import concourse.bacc as bacc
import concourse.bass as bass
import concourse.tile as tile
from concourse import mybir
from concourse.bass_utils import run_bass_kernel_spmd

B, N, M, D = 128, 512, 512, 128
NCORES = 8
BPC = B // NCORES  # batches per core
SQRT_D = float(np.sqrt(D))

F32 = mybir.dt.float32
F32R = mybir.dt.float32r
AF = mybir.ActivationFunctionType
OP = mybir.AluOpType

_prog_cache: dict = {}


def _f32(ap):
    return ap.bitcast(F32)


def _build(bpc: int, repeat: int = 1, cfg: dict | None = None):
    cfg = cfg or {}
    _pair = cfg.get("pair", False)
    ins_bufs = cfg.get("ins_bufs", 2 if _pair else 4)
    work_bufs = cfg.get("work_bufs", 2)
    outp_bufs = cfg.get("outp_bufs", 2)
    bias_banks = cfg.get("bias_banks", 1)  # m-chunks per bias PSUM tile
    sc_banks = cfg.get("sc_banks", 2)  # n-chunks per score PSUM tile
    nd_bufs = cfg.get("nd_bufs", 1)
    kv_bufs = cfg.get("kv_bufs", 1)
    q_bufs = cfg.get("q_bufs", 1)
    bias_bufs = cfg.get("bias_bufs", 2)
    sc_bufs = cfg.get("sc_bufs", 1)
    kvq_share = cfg.get("kvq_share", False)  # q reuses the kv PSUM slot
    bsum_gpsimd = cfg.get("bsum_gpsimd", False)
    probs_gpsimd = cfg.get("probs_gpsimd", False)
    # no_mask: compiled variant for the (checked at runtime) case
    # ninf_mask == 0 everywhere: mask DMA/adds drop out, output identical.
    no_mask = cfg.get("no_mask", False)
    sums_dve = cfg.get("sums_dve", True)  # softmax sums on DVE, not ACT accum
    fold_dve = cfg.get("fold_dve", False)  # -sqrt(D)*c2*cd via DVE stt, not PE
    probs_dma_split = cfg.get("probs_dma_split", 1)  # store in N pieces
    pair = cfg.get("pair", False)  # 2-batch DMA granularity (bigger transfers)

    nc = bacc.Bacc(
        "TRN2",
        target_bir_lowering=False,
        debug=False,
        num_devices=NCORES,
    )

    cd_d = nc.dram_tensor("cd", (bpc, N, M), F32R, kind="ExternalInput").ap()
    mask_d = (
        None
        if no_mask
        else nc.dram_tensor("maskd", (bpc, N, M), F32R, kind="ExternalInput").ap()
    )
    # encT and elnT ride in one tensor ([:, :, :M] / [:, :, M:]) so each
    # batch needs one 512KB aux DMA instead of two 256KB ones.
    aux_d = nc.dram_tensor("auxT", (bpc, D, M + N), F32R, kind="ExternalInput").ap()
    load_d = nc.dram_tensor("loadrow", (bpc, 1, N), F32R, kind="ExternalInput").ap()
    wkv_d = nc.dram_tensor("wkvT", (D, 2 * D), F32R, kind="ExternalInput").ap()
    wq1_d = nc.dram_tensor("wq1T", (D, D), F32R, kind="ExternalInput").ap()
    wq2_d = nc.dram_tensor("wq2", (1, D), F32R, kind="ExternalInput").ap()
    id1_d = nc.dram_tensor("id1", (128, 128), F32R, kind="ExternalInput").ap()
    idc2_d = nc.dram_tensor("idc2", (128, 128), F32R, kind="ExternalInput").ap()
    # per-partition scalars: [:, 0] = -c1 (ACT scale), [:, 1] = -0.1*c1
    # (un-scales the pre-divided mask in the logits step), [:, 2] = -sqrt(D)*c2
    scal_d = nc.dram_tensor("scal", (128, 4), F32, kind="ExternalInput").ap()
    probs_d = nc.dram_tensor("probs", (bpc, N, M), F32, kind="ExternalOutput").ap()

    with tile.TileContext(nc) as tc, ExitStack() as ctx:
        consts = ctx.enter_context(tc.tile_pool(name="consts", bufs=1))
        ins = ctx.enter_context(tc.tile_pool(name="ins", bufs=ins_bufs))
        work = ctx.enter_context(tc.tile_pool(name="work", bufs=work_bufs))
        outp = ctx.enter_context(tc.tile_pool(name="outp", bufs=outp_bufs))
        biasp = ctx.enter_context(
            tc.tile_pool(name="biasp", bufs=bias_bufs, space=bass.MemorySpace.PSUM)
        )
        kvp = ctx.enter_context(
            tc.tile_pool(name="kvp", bufs=kv_bufs, space=bass.MemorySpace.PSUM)
        )
        qp = ctx.enter_context(
            tc.tile_pool(name="qp", bufs=q_bufs, space=bass.MemorySpace.PSUM)
        )
        ndp = ctx.enter_context(
            tc.tile_pool(name="ndp", bufs=nd_bufs, space=bass.MemorySpace.PSUM)
        )
        scp = ctx.enter_context(
            tc.tile_pool(name="scp", bufs=sc_bufs, space=bass.MemorySpace.PSUM)
        )

        wkv_sb = consts.tile([D, 2 * D], F32R)
        nc.sync.dma_start(wkv_sb, wkv_d)
        wq1_sb = consts.tile([D, D], F32R)
        nc.sync.dma_start(wq1_sb, wq1_d)
        wq2_sb = consts.tile([1, D], F32R)
        nc.sync.dma_start(wq2_sb, wq2_d)
        id1_sb = consts.tile([128, 128], F32R)
        nc.sync.dma_start(id1_sb, id1_d)
        idc2_sb = consts.tile([128, 128], F32R)
        nc.sync.dma_start(idc2_sb, idc2_d)
        scal_sb = consts.tile([128, 4], F32)
        nc.sync.dma_start(scal_sb, scal_d)

        PAIRS = ((0, 1), (2, 3))
        seq = [b for _ in range(repeat) for b in range(bpc)]
        G = 2 if pair else 1
        for s0 in range(0, len(seq), G):
            grp = seq[s0 : s0 + G]
            b0 = grp[0]
            # n is mapped partition-major: SBUF slot (p, c) holds row
            # n = 4p + c, so each partition's DRAM footprint is 4 rows =
            # 8KB contiguous (fewer/larger DMA descriptors). elnT/loadrow
            # are host-permuted to the same n-slot ordering. With pair=True
            # two batches share one (2x bigger) DMA per tensor.
            cdg_t = ins.tile([128, G, 4, M], F32R, tag="cd")
            nc.sync.dma_start(
                cdg_t,
                cd_d[b0 : b0 + G].rearrange("b (p c) m -> p b c m", p=128),
            )
            if not no_mask:
                maskg_t = ins.tile([128, G, 4, M], F32R, tag="mask")
                nc.sync.dma_start(
                    maskg_t,
                    mask_d[b0 : b0 + G].rearrange("b (p c) m -> p b c m", p=128),
                )
            auxg_t = ins.tile([D, G, M + N], F32R, tag="auxT")
            nc.sync.dma_start(
                auxg_t, aux_d[b0 : b0 + G].rearrange("b d f -> d b f")
            )
            loadg_t = ins.tile([1, G, N], F32R, tag="load")
            nc.sync.dma_start(
                loadg_t, load_d[b0 : b0 + G].rearrange("b o n -> o b n")
            )
            probsg_t = outp.tile([128, G, 4, M], F32, tag="probs")
            for gi, b in enumerate(grp):
                cd_t = cdg_t[:, gi]
                mask_t = None if no_mask else maskg_t[:, gi]
                encT_t = auxg_t[:, gi, :M]
                elnT_t = auxg_t[:, gi, M:]
                load_t = loadg_t[:, gi]
                probs_t = probsg_t[:, gi]

                # ebT[m, n] = exp(-c1*(cd.T + mask'.T)) with mask' = mask/(-c1):
                # one DVE add fuses the two terms, then PE transposes (16 per
                # batch) move 128x128 tiles to m-on-partitions; ACT exp with
                # per-partition scale -c1 writes f32r ebT to SBUF.
                if no_mask:
                    bsum_t = cd_t
                else:
                    bsum_t = work.tile([128, 4, M], F32R, tag="bsum")
                    bsum_eng = nc.gpsimd if bsum_gpsimd else nc.vector
                    nsplit = cfg.get("bsum_split", 1)
                    step = 4 // nsplit
                    for c0 in range(0, 4, step):
                        bsum_eng.tensor_add(
                            bsum_t[:, c0 : c0 + step, :],
                            _f32(cd_t[:, c0 : c0 + step, :]),
                            _f32(mask_t[:, c0 : c0 + step, :]),
                        )
                ebT_t = work.tile([128, 4, N], F32R, tag="ebT")
                for g0 in range(0, 4, bias_banks):
                    bias_ps = biasp.tile([128, bias_banks, N], F32R, tag="bias")
                    for j in range(bias_banks):
                        mc = g0 + j
                        for c in range(4):
                            nc.tensor.matmul(
                                bias_ps[:, j, c * 128 : (c + 1) * 128],
                                bsum_t[:, c, mc * 128 : (mc + 1) * 128],
                                id1_sb,
                                start=True,
                                stop=True,
                                is_transpose=True,
                            )
                    nc.scalar.activation(
                        ebT_t[:, g0 : g0 + bias_banks, :],
                        _f32(bias_ps[:]),
                        AF.Exp,
                        scale=scal_sb[:, 0:1],
                    )

                # k|v per m-chunk; ek = exp(k), ekv = ek*v (m on partitions).
                ek_t = work.tile([128, 4, D], F32R, tag="ek")
                ekv_t = work.tile([128, 4, D], F32R, tag="ekv")
                for pair in PAIRS:
                    kv_ps = kvp.tile([128, 2, 2 * D], F32, tag="kv")
                    for j, mc in enumerate(pair):
                        nc.tensor.matmul(
                            kv_ps[:, j, :],
                            encT_t[:, mc * 128 : (mc + 1) * 128],
                            wkv_sb,
                            start=True,
                            stop=True,
                        )
                    nc.scalar.activation(
                        ek_t[:, pair[0] : pair[1] + 1, :], kv_ps[:, :, 0:D], AF.Exp
                    )
                    nc.vector.tensor_mul(
                        ekv_t[:, pair[0] : pair[1] + 1, :],
                        _f32(ek_t[:, pair[0] : pair[1] + 1, :]),
                        kv_ps[:, :, D : 2 * D],
                    )

                # qT[e, n] then sigmoid via tanh: sig = 0.5*tanh(q/2) + 0.5.
                if kvq_share:
                    q_ps = kvp.tile([128, N], F32, tag="kv")
                else:
                    q_ps = qp.tile([128, N], F32, tag="q")
                nc.tensor.matmul(q_ps, wq1_sb, elnT_t, start=True, stop=False)
                nc.tensor.matmul(q_ps, wq2_sb, load_t, start=False, stop=True)
                sig_t = work.tile([128, N], F32, tag="sig")
                nc.scalar.activation(sig_t, q_ps, AF.Tanh, scale=0.5)
                nc.vector.tensor_scalar(sig_t, sig_t, 0.5, 0.5, OP.mult, OP.add)

                # numT/denT[d, n] = (ekv|ek).T @ ebT, contracting m in 4 chunks.
                nd_ps = ndp.tile([128, 2, N], F32, tag="nd")
                for mc in range(4):
                    nc.tensor.matmul(
                        nd_ps[:, 0, :],
                        ekv_t[:, mc, :],
                        ebT_t[:, mc, :],
                        start=(mc == 0),
                        stop=(mc == 3),
                    )
                for mc in range(4):
                    nc.tensor.matmul(
                        nd_ps[:, 1, :],
                        ek_t[:, mc, :],
                        ebT_t[:, mc, :],
                        start=(mc == 0),
                        stop=(mc == 3),
                    )

                # aafmT = sig * num/max(den, tiny)  (tiny clamp mirrors
                # nan_to_num for fully-masked rows: num=0 -> 0).
                den_t = work.tile([128, N], F32, tag="den")
                nc.vector.tensor_scalar_max(den_t, nd_ps[:, 1, :], 1e-35)
                rden_t = work.tile([128, N], F32, tag="rden")
                nc.vector.reciprocal_approx_fast(rden_t, den_t)
                wr_t = work.tile([128, N], F32, tag="wr")
                nc.vector.tensor_mul(wr_t, nd_ps[:, 0, :], rden_t)
                aafm_t = work.tile([128, N], F32R, tag="aafm")
                nc.vector.tensor_mul(aafm_t, sig_t, wr_t)

                # score[n, m] = aafmT.T @ encT, plus -sqrt(D)*c2*cd folded in via
                # scaled identity; tanh(score/sqrt(D) - c2*cd), +mask, exp, softmax.
                exp_t = outp.tile([128, 4, M], F32, tag="exp")
                sums_t = outp.tile([128, 4], F32, tag="sums")
                for g0 in range(0, 4, sc_banks):
                    sc_ps = scp.tile([128, sc_banks, M], F32, tag="sc")
                    for j in range(sc_banks):
                        nt = g0 + j
                        nc.tensor.matmul(
                            sc_ps[:, j, :],
                            aafm_t[:, nt * 128 : (nt + 1) * 128],
                            encT_t,
                            start=True,
                            stop=fold_dve,
                        )
                        if not fold_dve:
                            nc.tensor.matmul(
                                sc_ps[:, j, :],
                                idc2_sb,
                                cd_t[:, nt, :],
                                start=False,
                                stop=True,
                            )
                    if fold_dve:
                        t0_t = work.tile([128, sc_banks, M], F32, tag="t0")
                        nc.vector.scalar_tensor_tensor(
                            t0_t,
                            _f32(cd_t[:, g0 : g0 + sc_banks, :]),
                            scal_sb[:, 2:3],
                            sc_ps[:],
                            OP.mult,
                            OP.add,
                        )
                        tanh_in = t0_t
                    else:
                        tanh_in = sc_ps[:]
                    h_t = work.tile([128, sc_banks, M], F32, tag="h")
                    nc.scalar.activation(h_t, tanh_in, AF.Tanh, scale=1.0 / SQRT_D)
                    if no_mask:
                        u_t = h_t
                    else:
                        # u = h + 0.1*mask = h + (-0.1*c1)*mask'
                        u_t = work.tile([128, sc_banks, M], F32, tag="u")
                        nc.vector.scalar_tensor_tensor(
                            u_t,
                            _f32(mask_t[:, g0 : g0 + sc_banks, :]),
                            scal_sb[:, 1:2],
                            h_t,
                            OP.mult,
                            OP.add,
                        )
                    for j in range(sc_banks):
                        nt = g0 + j
                        nc.scalar.activation(
                            exp_t[:, nt, :],
                            u_t[:, j, :],
                            AF.Exp,
                            scale=10.0,
                            accum_out=None if sums_dve else sums_t[:, nt : nt + 1],
                        )
                    if sums_dve:
                        nc.vector.tensor_reduce(
                            sums_t[:, g0 : g0 + sc_banks],
                            exp_t[:, g0 : g0 + sc_banks, :],
                            axis=mybir.AxisListType.X,
                            op=OP.add,
                        )
                rsum_t = outp.tile([128, 4], F32, tag="rsum")
                nc.vector.reciprocal(rsum_t, sums_t)
                probs_eng = nc.gpsimd if probs_gpsimd else nc.vector
                for nt in range(4):
                    probs_eng.tensor_scalar_mul(
                        probs_t[:, nt, :], exp_t[:, nt, :], rsum_t[:, nt : nt + 1]
                    )
            nc.sync.dma_start(
                probs_d[b0 : b0 + G].rearrange("b (p c) m -> p b c m", p=128),
                probsg_t,
            )

    nc.compile()
    return nc


def _get_prog(bpc: int, repeat: int = 1, cfg: dict | None = None):
    key = (bpc, repeat, tuple(sorted((cfg or {}).items())))
    if key not in _prog_cache:
        _prog_cache[key] = _build(bpc, repeat, cfg)
    return _prog_cache[key]


def _make_in_maps(
    encoded_last_node,
    load,
    cur_dist,
    log_scale,
    ninf_mask,
    encoded_nodes,
    Wq_last,
    Wk,
    Wv,
    alpha1,
    alpha2,
    n_cores=NCORES,
):
    f = np.float32
    c1 = float(np.asarray(alpha1).reshape(-1)[0]) * float(np.asarray(log_scale))
    c2 = float(np.asarray(alpha2).reshape(-1)[0]) * float(np.asarray(log_scale))
    # mask is uploaded pre-divided by -c1 (see module docstring); clamp c1
    # away from 0 to keep that finite. Exact when mask == 0 or |c1| >= 1e-20.
    c1s = c1 if abs(c1) >= 1e-20 else (1e-20 if c1 >= 0 else -1e-20)

    cd = np.ascontiguousarray(np.asarray(cur_dist, f))
    mask_np = np.asarray(ninf_mask, f)
    no_mask = not np.any(mask_np)
    maskp = (
        None
        if no_mask
        else np.ascontiguousarray(mask_np / np.float32(-c1s))
    )
    encT = np.asarray(encoded_nodes, f).transpose(0, 2, 1)
    # n-slot permutation (slot j holds row 4*(j%128) + j//128) to match the
    # partition-major on-chip layout of cd/mask/probs.
    perm = 4 * (np.arange(N) % 128) + np.arange(N) // 128
    elnT = np.asarray(encoded_last_node, f).transpose(0, 2, 1)[:, :, perm]
    auxT = np.ascontiguousarray(np.concatenate([encT, elnT], axis=2))
    loadrow = np.ascontiguousarray(np.asarray(load, f)[:, perm].reshape(B, 1, N))

    Wq = np.asarray(Wq_last, f)
    wkvT = np.ascontiguousarray(
        np.concatenate([np.asarray(Wk, f).T, np.asarray(Wv, f).T], axis=1)
    )
    wq1T = np.ascontiguousarray(Wq[:, :D].T)
    wq2 = np.ascontiguousarray(Wq[:, D : D + 1].T)

    eye = np.eye(128, dtype=f)
    scal = np.zeros((128, 4), f)
    scal[:, 0] = -c1s
    scal[:, 1] = -0.1 * c1s
    scal[:, 2] = -SQRT_D * c2
    shared = {
        "wkvT": wkvT,
        "wq1T": wq1T,
        "wq2": wq2,
        "id1": eye,
        "idc2": np.ascontiguousarray((-SQRT_D * c2) * eye),
        "scal": scal,
    }

    bpc = B // n_cores
    in_maps = []
    for i in range(n_cores):
        sl = slice(i * bpc, (i + 1) * bpc)
        m = {
            "cd": cd[sl],
            "auxT": auxT[sl],
            "loadrow": loadrow[sl],
            **shared,
        }
        if not no_mask:
            m["maskd"] = maskp[sl]
        in_maps.append(m)
    return in_maps, no_mask


def _run(trace=False, repeat=1, cfg=None, **inputs):
    """Build + run on 8 cores; returns (probs, BassKernelResults)."""
    in_maps, no_mask = _make_in_maps(**inputs)
    cfg = dict(cfg or {})
    cfg["no_mask"] = no_mask
    nc = _get_prog(BPC, repeat, cfg)
    res = run_bass_kernel_spmd(nc, in_maps, core_ids=list(range(NCORES)), trace=trace)
    probs = np.concatenate([r["probs"] for r in res.results], axis=0)
    return np.ascontiguousarray(probs.astype(np.float32)), res


def kernel(**inputs) -> np.ndarray:
    probs, _ = _run(trace=False, **inputs)
    return probs


if __name__ == "__main__":
    rng = np.random.default_rng(0)
    demo = {
        "encoded_last_node": rng.standard_normal((B, N, D), dtype=np.float32),
        "load": rng.random((B, N), dtype=np.float32),
        "cur_dist": rng.random((B, N, M), dtype=np.float32),
        "log_scale": np.ones((), np.float32),
        "ninf_mask": np.zeros((B, N, M), np.float32),
        "encoded_nodes": rng.standard_normal((B, M, D), dtype=np.float32),
        "Wq_last": rng.standard_normal((D, D + 1), dtype=np.float32) / SQRT_D,
        "Wk": rng.standard_normal((D, D), dtype=np.float32) / SQRT_D,
        "Wv": rng.standard_normal((D, D), dtype=np.float32) / SQRT_D,
        "alpha1": np.ones((1,), np.float32),
        "alpha2": np.ones((1,), np.float32),
    }
    out = kernel(**demo)
    print("kernel output", out.shape, out.dtype, out.sum())

